# revision 1
# baseline (speedup 1.0000x reference)
"""Trainium2 Bass kernel for the DDI DEDICOM decoder (nn_DDI_dedicom).

Reference computation (per edge a, relation b):
    x1 = x[edge[0]], x2 = x[edge[1]]                       # gather  [E, IN]
    row = BN(x1 @ W.T + b), col = BN(x2 @ W.T + b)         # linear + global-batch BN
    out[a, b] = sigmoid(row_a^T  diag(D_b) R diag(D_b)  col_a)

Sharding: data-parallel over E across 8 cores (E_s = E/8 = 4096 per core).
x / weights / R / D replicated. BatchNorm statistics are global over E:
each core computes per-feature partial sums (sum, sumsq) of its shard's
linear outputs; a [128,4] AllReduce produces the global stats.

Device layout is feature-major ([128 features on partitions, edges on free
dim]) so that the linear and the 16 DEDICOM matmuls contract features on
the PE, BN stats are free-axis reductions, and BN application is a
per-partition scale/bias.  Gathered edge-major [128e, 128f] tiles are
transposed on the PE.  The final per-edge dot (sum_i row*u) is an
elementwise DVE multiply + a PE "selector" matmul ([128,16] one-hot
column b) accumulating all 16 relations into one [16, 512] PSUM tile.
Output is produced relation-major [16, E_s]; the host transposes while
unsharding.
"""

import sys

sys.path.insert(0, "/opt/trn_rl_repo")

import numpy as np

import concourse.bass as bass
import concourse.tile as tile
from concourse import bacc, mybir
from concourse.bass_utils import run_bass_kernel_spmd

# Problem sizes (hardcoded per contract)
N_NODES = 50000
E = 32768
IN_DIM = 128
HID = 128
OUT = 16
EPS = 1e-5
N_CORES = 8
E_S = E // N_CORES          # 4096 edges per core
J = E_S // 128              # 32 gather blocks per side
NCH = E_S // 512            # 8 free-dim chunks of 512

F32 = mybir.dt.float32
F32R = mybir.dt.float32r

def _build(stage=3):
    """stage: 0=gather+linear, 1=+stats/cc/BN, 2=+dedicom(no out dbg), 3=full."""
    nc = bacc.Bacc(None, target_bir_lowering=False, debug=False, num_devices=N_CORES)

    # ---- I/O ----
    x = nc.dram_tensor("x", [N_NODES, IN_DIM], F32, kind="ExternalInput")
    idx1 = nc.dram_tensor("idx1", [128, J], mybir.dt.int32, kind="ExternalInput")
    idx2 = nc.dram_tensor("idx2", [128, J], mybir.dt.int32, kind="ExternalInput")
    w_t = nc.dram_tensor("w_t", [IN_DIM, HID], F32, kind="ExternalInput")
    r_t = nc.dram_tensor("r_t", [HID, HID], F32, kind="ExternalInput")
    d_m = nc.dram_tensor("d_m", [OUT, HID], F32, kind="ExternalInput")
    d_t = nc.dram_tensor("d_t", [HID, OUT], F32, kind="ExternalInput")
    lin_b = nc.dram_tensor("lin_b", [HID, 1], F32, kind="ExternalInput")
    gamma = nc.dram_tensor("gamma", [HID, 1], F32, kind="ExternalInput")
    beta = nc.dram_tensor("beta", [HID, 1], F32, kind="ExternalInput")
    ident = nc.dram_tensor("ident", [128, 128], F32, kind="ExternalInput")
    sel = nc.dram_tensor("sel", [128, OUT, OUT], F32, kind="ExternalInput")
    out = nc.dram_tensor("out", [OUT, E_S], F32, kind="ExternalOutput")
    if stage <= 1:
        row_dbg = nc.dram_tensor("row_dbg", [HID, E_S], F32, kind="ExternalOutput")
        col_dbg = nc.dram_tensor("col_dbg", [HID, E_S], F32, kind="ExternalOutput")

    with tile.TileContext(nc) as tc:
        with (
            tc.tile_pool(name="dramp", bufs=1, space="DRAM") as dramp,
            tc.tile_pool(name="consts", bufs=1) as consts,
            tc.tile_pool(name="gat", bufs=8) as gat,
            tc.tile_pool(name="big", bufs=1) as big,
            tc.tile_pool(name="zs", bufs=6) as zs,
            tc.tile_pool(name="small", bufs=2) as small,
            tc.tile_pool(name="outp", bufs=2) as outp,
            tc.tile_pool(name="psU", bufs=5, space="PSUM") as psU,
            tc.tile_pool(name="psO", bufs=3, space="PSUM") as psO,
        ):
            # ---- constants ----
            # idx first: the gather stream is the front-phase critical path
            idx1_s = consts.tile([128, J], mybir.dt.int32)
            nc.sync.dma_start(out=idx1_s[:], in_=idx1[:])
            idx2_s = consts.tile([128, J], mybir.dt.int32)
            nc.sync.dma_start(out=idx2_s[:], in_=idx2[:])
            w_t_s = consts.tile([IN_DIM, HID], F32)
            nc.sync.dma_start(out=w_t_s[:], in_=w_t[:])
            ident_s = consts.tile([128, 128], F32)
            nc.sync.dma_start(out=ident_s[:], in_=ident[:])
            r_t_s = consts.tile([HID, HID], F32)
            nc.sync.dma_start(out=r_t_s[:], in_=r_t[:])
            d_t_s = consts.tile([HID, OUT], F32)
            nc.sync.dma_start(out=d_t_s[:], in_=d_t[:])
            sel_s = consts.tile([128, OUT, OUT], F32)
            nc.sync.dma_start(out=sel_s[:], in_=sel[:])
            lin_b_s = consts.tile([HID, 1], F32)
            nc.sync.dma_start(out=lin_b_s[:], in_=lin_b[:])
            gamma_s = consts.tile([HID, 1], F32)
            nc.sync.dma_start(out=gamma_s[:], in_=gamma[:])
            beta_s = consts.tile([HID, 1], F32)
            nc.sync.dma_start(out=beta_s[:], in_=beta[:])
            # D broadcast across partitions: dbc[p, b, i] = D[b, i]
            dbc_s = consts.tile([128, OUT, HID], F32)
            nc.sync.dma_start(
                out=dbc_s[:],
                in_=bass.AP(tensor=d_m, offset=0, ap=[[0, 128], [HID, OUT], [1, HID]]),
            )
            eps_s = consts.tile([HID, 1], F32)
            nc.vector.memset(eps_s[:], EPS)
            # fp32r-rounded copy of the selector weights
            sel_r = consts.tile([128, OUT, OUT], F32R)
            nc.vector.tensor_copy(out=sel_r[:], in_=sel_s[:])
            # centered R^T: R~U(0,1); moving the 0.5*J rank-1 part to an
            # exact fp32 path shrinks the f32r residual magnitudes ~10x
            r_c = consts.tile([HID, HID], F32)
            nc.vector.tensor_scalar_add(out=r_c[:], in0=r_t_s[:], scalar1=-0.5)

            # ---- per-side gather + transpose + linear + stats ----
            # side 0: linear psum->sbuf copies on ACT with accum_out -> y sums.
            # side 1: transpose copies on ACT with accum_out -> x sums (the y
            # sum is then W @ xsum + E_s*b, one tiny matvec), linear copies on
            # DVE.  This keeps the last gather -> stats -> collective chain
            # short: nothing expensive serializes after the gather stream ends.
            yTs = []
            sum_parts = []
            sq_parts = []
            xsum_sb = []
            for side, idx_s in ((0, idx1_s), (1, idx2_s)):
                xT = big.tile([128, E_S], F32, tag=f"xT{side}")
                xs_part = small.tile([128, J], F32, tag=f"xs{side}")
                yT = big.tile([128, E_S], F32, tag=f"yT{side}")
                s_part = small.tile([128, NCH], F32, tag=f"sum{side}")
                q_part = small.tile([128, NCH], F32, tag=f"sq{side}")

                def emit_linear_chunk(n, side=side, xT=xT, yT=yT, s_part=s_part,
                                      q_part=q_part):
                    sl = slice(n * 512, (n + 1) * 512)
                    yp = psU.tile([128, 512], F32, tag="u")
                    nc.tensor.matmul(
                        out=yp[:], lhsT=w_t_s[:], rhs=xT[:, sl], start=True, stop=True
                    )
                    if side == 0:
                        # psum -> sbuf with bias add + free-axis sum
                        nc.scalar.activation(
                            out=yT[:, sl],
                            in_=yp[:],
                            func=mybir.ActivationFunctionType.Identity,
                            bias=lin_b_s[:, 0:1],
                            scale=1.0,
                            accum_out=(s_part[:, n : n + 1] if stage >= 1 else None),
                        )
                    else:
                        nc.vector.tensor_scalar_add(
                            out=yT[:, sl], in0=yp[:], scalar1=lin_b_s[:, 0:1]
                        )
                    if stage >= 1:
                        sq = zs.tile([128, 512], F32, tag="sq_scratch")
                        nc.scalar.activation(
                            out=sq[:],
                            in_=yT[:, sl],
                            func=mybir.ActivationFunctionType.Square,
                            accum_out=q_part[:, n : n + 1],
                        )

                for j in range(J):
                    g = gat.tile([128, 128], F32, tag="g")
                    nc.gpsimd.indirect_dma_start(
                        out=g[:],
                        out_offset=None,
                        in_=x[:],
                        in_offset=bass.IndirectOffsetOnAxis(
                            ap=idx_s[:, j : j + 1], axis=0
                        ),
                    )
                    tp = psO.tile([128, 128], F32, tag="o")
                    nc.tensor.transpose(out=tp[:], in_=g[:], identity=ident_s[:])
                    if side == 1 and stage >= 1:
                        nc.scalar.activation(
                            out=xT[:, j * 128 : (j + 1) * 128],
                            in_=tp[:],
                            func=mybir.ActivationFunctionType.Copy,
                            accum_out=xs_part[:, j : j + 1],
                        )
                    else:
                        nc.vector.tensor_copy(
                            out=xT[:, j * 128 : (j + 1) * 128], in_=tp[:]
                        )
                    # emit the linear for a 512-chunk as soon as its 4 blocks
                    # are in; keeps the linear off the post-gather tail
                    if j % 4 == 3:
                        emit_linear_chunk(j // 4)
                yTs.append(yT)
                sum_parts.append(s_part)
                sq_parts.append(q_part)
                if side == 1 and stage >= 1:
                    xs1 = small.tile([128, 1], F32, tag="xs1r")
                    nc.vector.reduce_sum(
                        out=xs1[:], in_=xs_part[:], axis=mybir.AxisListType.X,
                        op=mybir.AluOpType.add,
                    )
                    ysum_ps = psU.tile([128, 1], F32, tag="u")
                    nc.tensor.matmul(
                        out=ysum_ps[:], lhsT=w_t_s[:], rhs=xs1[:], start=True,
                        stop=True,
                    )
                    ysum1 = small.tile([128, 1], F32, tag="ys1")
                    nc.vector.scalar_tensor_tensor(
                        out=ysum1[:],
                        in0=lin_b_s[:, 0:1],
                        scalar=float(E_S),
                        in1=ysum_ps[:],
                        op0=mybir.AluOpType.mult,
                        op1=mybir.AluOpType.add,
                    )
                    xsum_sb.append(ysum1)

            if stage == 0:
                nc.sync.dma_start(out=row_dbg[:], in_=yTs[0][:])
                nc.sync.dma_start(out=col_dbg[:], in_=yTs[1][:])

            if stage >= 1:
                # ---- pack partial stats + AllReduce ----
                stats_l = small.tile([128, 4], F32, tag="stats")
                for k, part in ((0, sum_parts[0]), (1, sq_parts[0]), (3, sq_parts[1])):
                    nc.vector.reduce_sum(
                        out=stats_l[:, k : k + 1],
                        in_=part[:],
                        axis=mybir.AxisListType.X,
                        op=mybir.AluOpType.add,
                    )
                nc.vector.tensor_copy(out=stats_l[:, 2:3], in_=xsum_sb[0][:])
                cc_in = dramp.tile([HID, 4], F32)
                cc_out = dramp.tile([HID, 4], F32, addr_space="Shared")
                nc.sync.dma_start(out=cc_in[:], in_=stats_l[:])
                nc.gpsimd.collective_compute(
                    "AllReduce",
                    mybir.AluOpType.add,
                    replica_groups=[list(range(N_CORES))],
                    ins=[cc_in[:]],
                    outs=[cc_out[:]],
                )
                stats_g = small.tile([128, 4], F32, tag="statsg")
                nc.sync.dma_start(out=stats_g[:], in_=cc_out[:])

                # PE keep-warm across the collective: WAW-serialized dummy
                # transposes (each ~0.3us) so the HAM clock gate stays at
                # 8/8 and the post-collective matmuls start at full rate
                warm_ps = psU.tile([128, 128], F32, tag="u")
                for _k in range(90):
                    nc.tensor.transpose(
                        out=warm_ps[:], in_=yTs[0][:, 0:128], identity=ident_s[:]
                    )

                # ---- build S_b^T tiles (independent of stats; fills cc bubble)
                s_all = big.tile([128, OUT, HID], F32R, tag="s_all")
                for b in range(OUT):
                    nc.vector.tensor_tensor(
                        out=s_all[:, b, :],
                        in0=r_c[:],
                        in1=dbc_s[:, b, :],
                        op=mybir.AluOpType.mult,
                    )
                    nc.vector.tensor_scalar_mul(
                        out=s_all[:, b, :],
                        in0=s_all[:, b, :],
                        scalar1=d_t_s[:, b : b + 1],
                    )

                # ---- finalize BN factors ----
                inv_e = 1.0 / float(E)
                bn_s = []
                bn_t = []
                for side in (0, 1):
                    mean = small.tile([128, 1], F32, tag=f"m{side}")
                    nc.scalar.mul(
                        out=mean[:], in_=stats_g[:, 2 * side : 2 * side + 1], mul=inv_e
                    )
                    ey2 = small.tile([128, 1], F32, tag=f"e2{side}")
                    nc.scalar.mul(
                        out=ey2[:],
                        in_=stats_g[:, 2 * side + 1 : 2 * side + 2],
                        mul=inv_e,
                    )
                    var = small.tile([128, 1], F32, tag=f"v{side}")
                    nc.vector.tensor_tensor(
                        out=var[:], in0=mean[:], in1=mean[:], op=mybir.AluOpType.mult
                    )
                    nc.vector.tensor_sub(out=var[:], in0=ey2[:], in1=var[:])
                    std = small.tile([128, 1], F32, tag=f"sd{side}")
                    nc.scalar.activation(
                        out=std[:],
                        in_=var[:],
                        func=mybir.ActivationFunctionType.Sqrt,
                        bias=eps_s[:, 0:1],
                        scale=1.0,
                    )
                    inv = small.tile([128, 1], F32, tag=f"iv{side}")
                    nc.vector.reciprocal(out=inv[:], in_=std[:])
                    sc = small.tile([128, 1], F32, tag=f"sc{side}")
                    nc.vector.tensor_tensor(
                        out=sc[:], in0=gamma_s[:], in1=inv[:], op=mybir.AluOpType.mult
                    )
                    sh = small.tile([128, 1], F32, tag=f"sh{side}")
                    nc.vector.tensor_tensor(
                        out=sh[:], in0=mean[:], in1=sc[:], op=mybir.AluOpType.mult
                    )
                    nc.vector.tensor_sub(out=sh[:], in0=beta_s[:], in1=sh[:])
                    bn_s.append(sc)
                    bn_t.append(sh)

                # ---- apply BN (feature-major: per-partition scale+shift) ----
                rowT = big.tile([128, E_S], F32, tag="rowT")
                colT = big.tile([128, E_S], F32R, tag="colT")
                colF = big.tile([128, E_S], F32, tag="colF")
                for dst, src, side, eng in (
                    (rowT, yTs[0], 0, "act"),
                    (colT, yTs[1], 1, "dve"),
                    (colF, yTs[1], 1, "dve"),
                ):
                    for n in range(NCH):
                        sl = slice(n * 512, (n + 1) * 512)
                        if eng == "act":
                            nc.scalar.activation(
                                out=dst[:, sl],
                                in_=src[:, sl],
                                func=mybir.ActivationFunctionType.Identity,
                                bias=bn_t[side][:, 0:1],
                                scale=bn_s[side][:, 0:1],
                            )
                        else:
                            nc.vector.tensor_scalar(
                                out=dst[:, sl],
                                in0=src[:, sl],
                                scalar1=bn_s[side][:, 0:1],
                                scalar2=bn_t[side][:, 0:1],
                                op0=mybir.AluOpType.mult,
                                op1=mybir.AluOpType.add,
                            )

                if stage == 1:
                    nc.sync.dma_start(out=row_dbg[:], in_=rowT[:])
                    nc.sync.dma_start(out=col_dbg[:], in_=colT[:])

            if stage >= 2:
                # ---- exact rank-1 branch: v += 0.5 * (row.D_b)(D_b.col) ----
                # (R = 0.5*J + Rc; the 0.5*J part factorizes and is computed
                # here in full fp32 so the f32r residual path only carries
                # the small centered magnitudes)
                pq_sb = big.tile([OUT, E_S], F32, tag="pq")

                # ---- DEDICOM residual: u = Sc_b^T.T @ colT ; z = rowT*u ----
                # software-pipelined: u-matmuls + z-muls run G steps ahead of
                # the strictly-ordered o-accumulation matmuls so the z latency
                # (DVE/ACT/GPSIMD) stays off the PE's critical path.
                G = 3
                for n in range(NCH):
                    sl = slice(n * 512, (n + 1) * 512)
                    # exact rank-1 branch for this chunk (fp32)
                    p_ps = psO.tile([OUT, 512], F32, tag="o")
                    nc.tensor.matmul(
                        out=p_ps[:], lhsT=d_t_s[:], rhs=rowT[:, sl],
                        start=True, stop=True,
                    )
                    q_ps = psO.tile([OUT, 512], F32, tag="o")
                    nc.tensor.matmul(
                        out=q_ps[:], lhsT=d_t_s[:], rhs=colF[:, sl],
                        start=True, stop=True,
                    )
                    q_sb = outp.tile([OUT, 512], F32, tag="qsb")
                    nc.scalar.copy(out=q_sb[:], in_=q_ps[:])
                    nc.vector.scalar_tensor_tensor(
                        out=pq_sb[:, sl],
                        in0=p_ps[:],
                        scalar=0.5,
                        in1=q_sb[:],
                        op0=mybir.AluOpType.mult,
                        op1=mybir.AluOpType.mult,
                    )
                    op_ = psO.tile([OUT, 512], F32, tag="o")
                    ztiles = [None] * OUT

                    def emit_u_z(b):
                        up = psU.tile([128, 512], F32, tag="u")
                        nc.tensor.matmul(
                            out=up[:],
                            lhsT=s_all[:, b, :],
                            rhs=colT[:, sl],
                            start=True,
                            stop=True,
                        )
                        z = zs.tile([128, 512], F32R, tag="z")
                        if b % 16 < 6:
                            # third lane: ACT copies PSUM->SBUF, GPSIMD muls
                            u_sb = zs.tile([128, 512], F32, tag="usb")
                            nc.scalar.copy(out=u_sb[:], in_=up[:])
                            nc.gpsimd.tensor_tensor(
                                out=z[:],
                                in0=u_sb[:],
                                in1=rowT[:, sl],
                                op=mybir.AluOpType.mult,
                            )
                        else:
                            nc.vector.tensor_tensor(
                                out=z[:],
                                in0=up[:],
                                in1=rowT[:, sl],
                                op=mybir.AluOpType.mult,
                            )
                        ztiles[b] = z

                    def emit_o(b):
                        nc.tensor.matmul(
                            out=op_[:],
                            lhsT=sel_r[:, b, :],
                            rhs=ztiles[b][:],
                            start=(b == 0),
                            stop=(b == OUT - 1),
                        )

                    for b in range(OUT):
                        emit_u_z(b)
                        if b >= G:
                            emit_o(b - G)
                    for b in range(OUT - G, OUT):
                        emit_o(b)
                    o_mg = outp.tile([OUT, 512], F32, tag="omg")
                    nc.vector.tensor_add(
                        out=o_mg[:], in0=op_[:], in1=pq_sb[:, sl]
                    )
                    o_sb = outp.tile([OUT, 512], F32, tag="osb")
                    nc.scalar.activation(
                        out=o_sb[:],
                        in_=o_mg[:],
                        func=mybir.ActivationFunctionType.Sigmoid,
                    )
                    nc.sync.dma_start(out=out[:, sl], in_=o_sb[:])

    nc.compile()
    return nc


_CACHE = {}


def _get_nc():
    if "nc" not in _CACHE:
        _CACHE["nc"] = _build()
    return _CACHE["nc"]


def _marshal(x, target_edge_index, lin_w, lin_b, bn_gamma, bn_beta, R, D):
    x = np.ascontiguousarray(np.asarray(x, dtype=np.float32))
    edges = np.asarray(target_edge_index)
    sel = np.zeros((128, OUT, OUT), dtype=np.float32)
    for b in range(OUT):
        sel[:, b, b] = 1.0
    common = {
        "w_t": np.ascontiguousarray(np.asarray(lin_w, np.float32).T),
        "r_t": np.ascontiguousarray(np.asarray(R, np.float32).T),
        "d_m": np.ascontiguousarray(np.asarray(D, np.float32)),
        "d_t": np.ascontiguousarray(np.asarray(D, np.float32).T),
        "lin_b": np.ascontiguousarray(np.asarray(lin_b, np.float32).reshape(HID, 1)),
        "gamma": np.ascontiguousarray(np.asarray(bn_gamma, np.float32).reshape(HID, 1)),
        "beta": np.ascontiguousarray(np.asarray(bn_beta, np.float32).reshape(HID, 1)),
        "ident": np.eye(128, dtype=np.float32),
        "sel": sel,
        "x": x,
    }
    in_maps = []
    for c in range(N_CORES):
        sl = slice(c * E_S, (c + 1) * E_S)
        i1 = edges[0, sl].astype(np.int32).reshape(J, 128).T
        i2 = edges[1, sl].astype(np.int32).reshape(J, 128).T
        in_maps.append(
            {**common, "idx1": np.ascontiguousarray(i1), "idx2": np.ascontiguousarray(i2)}
        )
    return in_maps


def kernel(x, target_edge_index, lin_w, lin_b, bn_gamma, bn_beta, R, D):
    nc = _get_nc()
    in_maps = _marshal(x, target_edge_index, lin_w, lin_b, bn_gamma, bn_beta, R, D)
    _CACHE["in_maps"] = in_maps
    res = run_bass_kernel_spmd(nc, in_maps, list(range(N_CORES)))
    shards = [res.results[c]["out"] for c in range(N_CORES)]  # each [16, E_S]
    full = np.concatenate(shards, axis=1)  # [16, E]
    return np.ascontiguousarray(full.T)  # [E, 16] float32



# revision 62
# speedup vs baseline: 1.1215x; 1.1215x over previous
"""Trainium2 Bass kernel for the DDI DEDICOM decoder (nn_DDI_dedicom).

Reference computation (per edge a, relation b):
    x1 = x[edge[0]], x2 = x[edge[1]]                       # gather  [E, IN]
    row = BN(x1 @ W.T + b), col = BN(x2 @ W.T + b)         # linear + global-batch BN
    out[a, b] = sigmoid(row_a^T  diag(D_b) R diag(D_b)  col_a)

Sharding: data-parallel over E across 8 cores (E_s = E/8 = 4096 per core).
x / weights / R / D replicated.  BatchNorm statistics are global over E:
each core computes per-feature partials (x-sum per side + sum of y^2 per
side), packed as a [128,4] tile; an AllGather (cheaper than AllReduce in
both the cost model and on the wire) + local tree-reduce produces the
global stats.  The y-sum is recovered as W @ xsum + E*b (exact).

Device layout is feature-major ([128 features on partitions, edges on the
free dim]): the linear and the 16 DEDICOM matmuls contract features on
the PE, BN stats are free-axis reductions, and BN application is a
per-partition scale/bias.  Gathers are batched 4 blocks per indirect DMA
(the v1 DMA cost has a 500ns/instruction floor).  Gathered edge-major
tiles are transposed on the PE with a bf16 identity (1 cycle/row).  The
final per-edge dot (sum_i row*u) is an elementwise multiply (split
DVE / ACT+GPSIMD lanes) + a PE "selector" matmul ([128,16] one-hot
column b) accumulating all 16 relations into one [16, 512] PSUM tile.
Output is produced relation-major [16, E_s]; the host transposes while
unsharding.
"""

import sys

sys.path.insert(0, "/opt/trn_rl_repo")

import numpy as np

import concourse.bass as bass
import concourse.tile as tile
from concourse import bacc, mybir
from concourse.bass_utils import run_bass_kernel_spmd

# Problem sizes (hardcoded per contract)
N_NODES = 50000
E = 32768
IN_DIM = 128
HID = 128
OUT = 16
EPS = 1e-5
N_CORES = 8
E_S = E // N_CORES          # 4096 edges per core
J = E_S // 128              # 32 gather blocks per side
NCH = E_S // 512            # 8 free-dim chunks of 512

F32 = mybir.dt.float32
F32R = mybir.dt.float32r
BF16 = mybir.dt.bfloat16

# tuning knobs
N_WARM = 82        # PE keep-warm matmuls spanning the collective bubble
CC_ALLGATHER = False  # AllGather+local reduce (False: plain AllReduce)
G = 5              # u/z software-pipeline depth ahead of the o-matmuls
POOL_LANES = frozenset(range(4, 12))  # z-lanes routed ACT-copy + GPSIMD-mul


def _build(centered=False):
    nc = bacc.Bacc(None, target_bir_lowering=False, debug=False, num_devices=N_CORES)

    # ---- I/O ----
    x = nc.dram_tensor("x", [N_NODES, IN_DIM], F32R, kind="ExternalInput")
    idx1 = nc.dram_tensor("idx1", [128, J], mybir.dt.int32, kind="ExternalInput")
    idx2 = nc.dram_tensor("idx2", [128, J], mybir.dt.int32, kind="ExternalInput")
    w_t = nc.dram_tensor("w_t", [IN_DIM, HID], F32R, kind="ExternalInput")
    r_t = nc.dram_tensor("r_t", [HID, HID], F32, kind="ExternalInput")
    d_m = nc.dram_tensor("d_m", [OUT, HID], F32, kind="ExternalInput")
    d_t = nc.dram_tensor("d_t", [HID, OUT], F32, kind="ExternalInput")
    lin_b = nc.dram_tensor("lin_b", [HID, 1], F32, kind="ExternalInput")
    gamma = nc.dram_tensor("gamma", [HID, 1], F32, kind="ExternalInput")
    beta = nc.dram_tensor("beta", [HID, 1], F32, kind="ExternalInput")
    ident = nc.dram_tensor("ident", [128, 128], F32, kind="ExternalInput")
    sel = nc.dram_tensor("sel", [128, OUT, OUT], F32, kind="ExternalInput")
    out = nc.dram_tensor("out", [OUT, E_S], F32, kind="ExternalOutput")

    with tile.TileContext(nc) as tc:
        with (
            tc.tile_pool(name="dramp", bufs=1, space="DRAM") as dramp,
            tc.tile_pool(name="consts", bufs=1) as consts,
            tc.tile_pool(name="gat", bufs=8) as gat,
            tc.tile_pool(name="big", bufs=1) as big,
            tc.tile_pool(name="zs", bufs=6) as zs,
            tc.tile_pool(name="small", bufs=2) as small,
            tc.tile_pool(name="outp", bufs=2) as outp,
            tc.tile_pool(name="psU", bufs=5, space="PSUM") as psU,
            tc.tile_pool(name="psO", bufs=3, space="PSUM") as psO,
        ):
            # ---- constants ----
            # idx first: the gather stream is the front-phase critical path
            idx1_s = consts.tile([128, J], mybir.dt.int32)
            nc.sync.dma_start(out=idx1_s[:], in_=idx1[:])
            idx2_s = consts.tile([128, J], mybir.dt.int32)
            nc.sync.dma_start(out=idx2_s[:], in_=idx2[:])
            w_t_s = consts.tile([IN_DIM, HID], F32R)
            nc.sync.dma_start(out=w_t_s[:], in_=w_t[:])
            ident_s = consts.tile([128, 128], F32)
            nc.sync.dma_start(out=ident_s[:], in_=ident[:])
            r_t_s = consts.tile([HID, HID], F32)
            nc.sync.dma_start(out=r_t_s[:], in_=r_t[:])
            d_t_s = consts.tile([HID, OUT], F32)
            nc.sync.dma_start(out=d_t_s[:], in_=d_t[:])
            sel_s = consts.tile([128, OUT, OUT], F32)
            nc.sync.dma_start(out=sel_s[:], in_=sel[:])
            lin_b_s = consts.tile([HID, 1], F32)
            nc.sync.dma_start(out=lin_b_s[:], in_=lin_b[:])
            gamma_s = consts.tile([HID, 1], F32)
            nc.sync.dma_start(out=gamma_s[:], in_=gamma[:])
            beta_s = consts.tile([HID, 1], F32)
            nc.sync.dma_start(out=beta_s[:], in_=beta[:])
            # D broadcast across partitions: dbc[p, b, i] = D[b, i]
            dbc_s = consts.tile([128, OUT, HID], F32)
            nc.sync.dma_start(
                out=dbc_s[:],
                in_=bass.AP(tensor=d_m, offset=0, ap=[[0, 128], [HID, OUT], [1, HID]]),
            )
            eps_s = consts.tile([HID, 1], F32)
            nc.vector.memset(eps_s[:], EPS)
            # f32r identity: the transpose's moving operand -> 1.5 cycles/row
            # (bf16 would be 1.0 but neuronxcc rejects mixed 32/16-bit matmuls)
            ident_b = consts.tile([128, 128], F32R)
            nc.vector.tensor_copy(out=ident_b[:], in_=ident_s[:])
            # fp32r-rounded copy of the selector weights
            sel_r = consts.tile([128, OUT, OUT], F32R)
            nc.vector.tensor_copy(out=sel_r[:], in_=sel_s[:])

            s_all = big.tile([128, OUT, HID], F32R, tag="s_all")

            def emit_s_all():
                # S_b^T tiles: s_all[f, b, j] = R[j,f] * D[b,j] * D[b,f].
                # Pure constants, but emitted on GPSIMD *after* the gather
                # stream so they can't steal front-end DVE/Pool bandwidth;
                # they complete long before the collective returns.
                # dbd[f, b, j] = D[b, f] * D[b, j] (in-place over dbc)
                nc.gpsimd.tensor_tensor(
                    out=dbc_s[:],
                    in0=dbc_s[:],
                    in1=bass.AP(
                        tensor=d_t_s.tensor,
                        offset=d_t_s[:].offset,
                        ap=[d_t_s[:].ap[0], [1, OUT], [0, HID]],
                    ),
                    op=mybir.AluOpType.mult,
                )
                nc.gpsimd.tensor_tensor(
                    out=s_all[:],
                    in0=bass.AP(
                        tensor=r_t_s.tensor,
                        offset=r_t_s[:].offset,
                        ap=[r_t_s[:].ap[0], [0, OUT], [1, HID]],
                    ),
                    in1=dbc_s[:],
                    op=mybir.AluOpType.mult,
                )

            # ---- per-side gather + transpose + linear + stats ----
            # The gather stream (64 single-offset indirect DMAs, 500ns floor
            # each on the Pool queue) is the front-end pacer; all per-chunk
            # elementwise work fits underneath it.  x-sums accumulate on the
            # ACT transpose-copies (y-sum is recovered post-collective as
            # W @ xsum_g + E*b); y^2 sums via ACT Square reading the linear's
            # PSUM with the bias folded in (lookbehind-1: zero queue stall).
            yTs = []
            xTs = []
            xs_parts = []
            sq_parts = []
            for side, idx_s in ((0, idx1_s), (1, idx2_s)):
                xT = big.tile([128, E_S], F32R, tag=f"xT{side}")
                yT = big.tile([128, E_S], F32, tag=f"yT{side}")
                xs_part = small.tile([128, NCH], F32, tag=f"xs{side}")
                q_part = small.tile([128, NCH], F32, tag=f"sq{side}")
                yps = []

                def emit_square(n, side=side, yps=None, q_part=q_part):
                    sq = zs.tile([128, 512], F32, tag="sq_scratch")
                    nc.scalar.activation(
                        out=sq[:],
                        in_=yps[n][:],
                        func=mybir.ActivationFunctionType.Square,
                        bias=lin_b_s[:, 0:1],
                        scale=1.0,
                        accum_out=q_part[:, n : n + 1],
                    )

                for n in range(NCH):  # 4 gathers == one 512 chunk
                    g4 = gat.tile([128, 4, 128], F32R, tag="g")
                    for k in range(4):
                        nc.gpsimd.indirect_dma_start(
                            out=g4[:, k, :],
                            out_offset=None,
                            in_=x[:],
                            in_offset=bass.IndirectOffsetOnAxis(
                                ap=idx_s[:, 4 * n + k : 4 * n + k + 1],
                                axis=0,
                            ),
                        )
                    tp4 = psU.tile([128, 4, 128], F32R, tag="u")
                    for k in range(4):
                        nc.tensor.transpose(
                            out=tp4[:, k, :], in_=g4[:, k, :], identity=ident_b[:]
                        )
                    sl = slice(n * 512, (n + 1) * 512)
                    # PSUM->SBUF with free-axis x-sum accumulation
                    nc.scalar.activation(
                        out=xT[:, sl],
                        in_=tp4[:],
                        func=mybir.ActivationFunctionType.Copy,
                        accum_out=xs_part[:, n : n + 1],
                    )
                    yp = psU.tile([128, 512], F32, tag="u")
                    nc.tensor.matmul(
                        out=yp[:], lhsT=w_t_s[:], rhs=xT[:, sl], start=True, stop=True
                    )
                    # psum -> sbuf with bias add (DVE)
                    nc.vector.tensor_scalar_add(
                        out=yT[:, sl], in0=yp[:], scalar1=lin_b_s[:, 0:1]
                    )
                    if n >= 1:
                        emit_square(n - 1, yps=yps)
                    yps.append(yp)
                yTs.append(yT)
                xTs.append(xT)
                xs_parts.append(xs_part)
                sq_parts.append(q_part)
                emit_square(NCH - 1, yps=yps)

            # constant S-tile build on GPSIMD after the gather stream drains
            emit_s_all()

            # ---- pack partial stats + collective ----
            # layout: [xsum0, xsum1, ysq0, ysq1] so the BN finalize can
            # process both sides with 2-column vector ops
            stats_l = small.tile([128, 4], F32, tag="stats")
            for k, part in (
                (0, xs_parts[0]),
                (1, xs_parts[1]),
                (2, sq_parts[0]),
                (3, sq_parts[1]),
            ):
                nc.vector.reduce_sum(
                    out=stats_l[:, k : k + 1],
                    in_=part[:],
                    axis=mybir.AxisListType.X,
                    op=mybir.AluOpType.add,
                )
            cc_in = dramp.tile([HID, 4], F32)
            if CC_ALLGATHER:
                cc_out = dramp.tile([N_CORES, HID, 4], F32, addr_space="Shared")
            else:
                cc_out = dramp.tile([HID, 4], F32, addr_space="Shared")
            nc.sync.dma_start(out=cc_in[:], in_=stats_l[:])
            # pre-load ACT tables for funcs first used after the collective
            # (Sigmoid / Rsqrt) while ACT idles in the bubble
            dum = small.tile([128, 1], F32, tag="dum")
            nc.scalar.activation(
                out=dum[:], in_=eps_s[:], func=mybir.ActivationFunctionType.Sigmoid
            )
            nc.scalar.activation(
                out=dum[:], in_=eps_s[:], func=mybir.ActivationFunctionType.Sqrt
            )
            if CC_ALLGATHER:
                nc.gpsimd.collective_compute(
                    "AllGather",
                    mybir.AluOpType.bypass,
                    replica_groups=[list(range(N_CORES))],
                    ins=[cc_in[:]],
                    outs=[cc_out[:]],
                )
                stats_r = small.tile([128, N_CORES, 4], F32, tag="statsr")
                nc.sync.dma_start(
                    out=stats_r[:],
                    in_=bass.AP(
                        tensor=cc_out.tensor,
                        offset=cc_out[:].offset,
                        ap=[[4, 128], [HID * 4, N_CORES], [1, 4]],
                    ),
                )
                # reduce the 8 ranks' partials in one strided op: view the
                # [128, 8, 4] tile as [128, 4, 8] (k outer / rank inner) so
                # the X-axis reduction sums over ranks
                stats_g = small.tile([128, 4], F32, tag="statsg")
                nc.vector.reduce_sum(
                    out=stats_g[:],
                    in_=bass.AP(
                        tensor=stats_r.tensor,
                        offset=stats_r[:].offset,
                        ap=[stats_r[:].ap[0], [1, 4], [4, N_CORES]],
                    ),
                    axis=mybir.AxisListType.X,
                    op=mybir.AluOpType.add,
                )
            else:
                nc.gpsimd.collective_compute(
                    "AllReduce",
                    mybir.AluOpType.add,
                    replica_groups=[list(range(N_CORES))],
                    ins=[cc_in[:]],
                    outs=[cc_out[:]],
                )
                stats_g = small.tile([128, 4], F32, tag="statsg")
                nc.sync.dma_start(out=stats_g[:], in_=cc_out[:])

            # PE keep-warm across the collective: WAW-serialized dummy
            # matmuls so the clock stays at full rate for the post-collective
            # DEDICOM matmuls
            warm_ps = psU.tile([128, 512], F32, tag="u")
            for _k in range(N_WARM):
                nc.tensor.matmul(
                    out=warm_ps[:], lhsT=w_t_s[:], rhs=xTs[0][:, 0:512],
                    start=True, stop=True,
                )

            # ---- finalize BN factors (both sides as [128, 2] vector ops) ----
            # means = (W @ xsum_g)/E + b   (exact: the linear is affine)
            inv_e = 1.0 / float(E)
            xsum2 = small.tile([128, 2], F32R, tag="xs2")
            nc.vector.tensor_copy(out=xsum2[:], in_=stats_g[:, 0:2])
            ysum_big = psU.tile([128, 512], F32, tag="u")
            nc.tensor.matmul(
                out=ysum_big[:, 0:2], lhsT=w_t_s[:], rhs=xsum2[:],
                start=True, stop=True,
            )
            means = small.tile([128, 2], F32, tag="means")
            nc.vector.scalar_tensor_tensor(
                out=means[:],
                in0=ysum_big[:, 0:2],
                scalar=inv_e,
                in1=bass.AP(
                    tensor=lin_b_s.tensor,
                    offset=lin_b_s[:].offset,
                    ap=[lin_b_s[:].ap[0], [0, 2]],
                ),
                op0=mybir.AluOpType.mult,
                op1=mybir.AluOpType.add,
            )
            sgs = small.tile([128, 2], F32, tag="sgs")  # [ey2_0, ey2_1]
            nc.scalar.mul(out=sgs[:], in_=stats_g[:, 2:4], mul=inv_e)
            m2 = small.tile([128, 2], F32, tag="m2")
            nc.vector.tensor_tensor(
                out=m2[:], in0=means[:], in1=means[:], op=mybir.AluOpType.mult
            )
            var2 = small.tile([128, 2], F32, tag="var2")
            nc.vector.tensor_sub(out=var2[:], in0=sgs[:], in1=m2[:])
            std2 = small.tile([128, 2], F32, tag="std2")
            nc.scalar.activation(
                out=std2[:],
                in_=var2[:],
                func=mybir.ActivationFunctionType.Sqrt,
                bias=eps_s[:, 0:1],
                scale=1.0,
            )
            rstd = small.tile([128, 2], F32, tag="rstd")
            nc.vector.reciprocal(out=rstd[:], in_=std2[:])
            bn_sc = small.tile([128, 2], F32, tag="bnsc")
            nc.vector.tensor_scalar_mul(
                out=bn_sc[:], in0=rstd[:], scalar1=gamma_s[:, 0:1]
            )
            msc = small.tile([128, 2], F32, tag="msc")
            nc.vector.tensor_tensor(
                out=msc[:], in0=means[:], in1=bn_sc[:], op=mybir.AluOpType.mult
            )
            bn_sh = small.tile([128, 2], F32, tag="bnsh")
            nc.vector.tensor_scalar(
                out=bn_sh[:],
                in0=msc[:],
                scalar1=-1.0,
                scalar2=beta_s[:, 0:1],
                op0=mybir.AluOpType.mult,
                op1=mybir.AluOpType.add,
            )
            bn_s = [bn_sc[:, 0:1], bn_sc[:, 1:2]]
            bn_t = [bn_sh[:, 0:1], bn_sh[:, 1:2]]

            # ---- apply BN (feature-major: per-partition scale+shift) ----
            # emitted just-in-time inside the DEDICOM loop (one chunk of
            # lookahead) so the 8-chunk BN batch can't head-of-line-block the
            # DVE/ACT queues at back-end start
            rowT = big.tile([128, E_S], F32R, tag="rowT")
            colT = big.tile([128, E_S], F32R, tag="colT")

            def emit_bn(n):
                sl = slice(n * 512, (n + 1) * 512)
                nc.vector.tensor_scalar(
                    out=colT[:, sl],
                    in0=yTs[1][:, sl],
                    scalar1=bn_s[1],
                    scalar2=bn_t[1],
                    op0=mybir.AluOpType.mult,
                    op1=mybir.AluOpType.add,
                )
                nc.scalar.activation(
                    out=rowT[:, sl],
                    in_=yTs[0][:, sl],
                    func=mybir.ActivationFunctionType.Identity,
                    bias=bn_t[0],
                    scale=bn_s[0],
                )

            emit_bn(0)

            # ---- DEDICOM: u_b = S_b^T col ; z = row*u ; o_b = sum_i z ----
            # one flat (chunk, b) pipeline: u-matmuls + z-muls run G steps
            # ahead of the strictly-ordered o-accumulation matmuls, ACROSS
            # chunk boundaries, so the z latency (DVE/ACT/GPSIMD) stays off
            # the PE's critical path even at chunk seams.
            op_tiles = [None] * NCH
            ztiles = [[None] * OUT for _ in range(NCH)]

            def emit_u_z(n, b):
                sl = slice(n * 512, (n + 1) * 512)
                if b == 0:
                    if n + 1 < NCH:
                        emit_bn(n + 1)
                    op_t = psO.tile([OUT, 512], F32, tag="o")
                    op_tiles[n] = op_t
                up = psU.tile([128, 512], F32, tag="u")
                nc.tensor.matmul(
                    out=up[:],
                    lhsT=s_all[:, b, :],
                    rhs=colT[:, sl],
                    start=True,
                    stop=True,
                )
                z = zs.tile([128, 512], F32R, tag="z")
                if b in POOL_LANES:
                    # ACT copies PSUM->SBUF, GPSIMD muls
                    u_sb = zs.tile([128, 512], F32, tag="usb")
                    nc.scalar.copy(out=u_sb[:], in_=up[:])
                    nc.gpsimd.tensor_tensor(
                        out=z[:],
                        in0=u_sb[:],
                        in1=rowT[:, sl],
                        op=mybir.AluOpType.mult,
                    )
                else:
                    nc.vector.tensor_tensor(
                        out=z[:],
                        in0=up[:],
                        in1=rowT[:, sl],
                        op=mybir.AluOpType.mult,
                    )
                ztiles[n][b] = z

            def emit_o(n, b):
                nc.tensor.matmul(
                    out=op_tiles[n][:],
                    lhsT=sel_r[:, b, :],
                    rhs=ztiles[n][b][:],
                    start=(b == 0),
                    stop=(b == OUT - 1),
                )
                ztiles[n][b] = None

            def emit_sig(n):
                sl = slice(n * 512, (n + 1) * 512)
                o_sb = outp.tile([OUT, 512], F32, tag="osb")
                nc.scalar.activation(
                    out=o_sb[:],
                    in_=op_tiles[n][:],
                    func=mybir.ActivationFunctionType.Sigmoid,
                )
                nc.sync.dma_start(out=out[:, sl], in_=o_sb[:])

            work = [(n, b) for n in range(NCH) for b in range(OUT)]
            for i, (n, b) in enumerate(work):
                emit_u_z(n, b)
                if i >= G:
                    emit_o(*work[i - G])
                # sigmoid for the previous chunk is emitted a few u_sb copies
                # into this one, so it can't head-of-line-block the ACT queue
                # while it waits on the previous chunk's last o-matmul
                if b == 8 and n > 0:
                    emit_sig(n - 1)
            for i in range(len(work) - G, len(work)):
                emit_o(*work[i])
            emit_sig(NCH - 1)

    nc.compile()
    return nc


_CACHE = {}


def _get_nc():
    if "nc" not in _CACHE:
        _CACHE["nc"] = _build()
    return _CACHE["nc"]


def _marshal(x, target_edge_index, lin_w, lin_b, bn_gamma, bn_beta, R, D):
    x = np.ascontiguousarray(np.asarray(x, dtype=np.float32))
    edges = np.asarray(target_edge_index)
    sel = np.zeros((128, OUT, OUT), dtype=np.float32)
    for b in range(OUT):
        sel[:, b, b] = 1.0
    common = {
        "w_t": np.ascontiguousarray(np.asarray(lin_w, np.float32).T),
        "r_t": np.ascontiguousarray(np.asarray(R, np.float32).T),
        "d_m": np.ascontiguousarray(np.asarray(D, np.float32)),
        "d_t": np.ascontiguousarray(np.asarray(D, np.float32).T),
        "lin_b": np.ascontiguousarray(np.asarray(lin_b, np.float32).reshape(HID, 1)),
        "gamma": np.ascontiguousarray(np.asarray(bn_gamma, np.float32).reshape(HID, 1)),
        "beta": np.ascontiguousarray(np.asarray(bn_beta, np.float32).reshape(HID, 1)),
        "ident": np.eye(128, dtype=np.float32),
        "sel": sel,
        "x": x,
    }
    in_maps = []
    for c in range(N_CORES):
        sl = slice(c * E_S, (c + 1) * E_S)
        i1 = edges[0, sl].astype(np.int32).reshape(J, 128).T
        i2 = edges[1, sl].astype(np.int32).reshape(J, 128).T
        in_maps.append(
            {**common, "idx1": np.ascontiguousarray(i1), "idx2": np.ascontiguousarray(i2)}
        )
    return in_maps


def kernel(x, target_edge_index, lin_w, lin_b, bn_gamma, bn_beta, R, D):
    nc = _get_nc()
    in_maps = _marshal(x, target_edge_index, lin_w, lin_b, bn_gamma, bn_beta, R, D)
    _CACHE["in_maps"] = in_maps
    res = run_bass_kernel_spmd(nc, in_maps, list(range(N_CORES)))
    shards = [res.results[c]["out"] for c in range(N_CORES)]  # each [16, E_S]
    full = np.concatenate(shards, axis=1)  # [16, E]
    return np.ascontiguousarray(full.T)  # [E, 16] float32


# revision 63
# speedup vs baseline: 1.2310x; 1.0977x over previous
"""Trainium2 Bass kernel for the DDI DEDICOM decoder (nn_DDI_dedicom).

Reference computation (per edge a, relation b):
    x1 = x[edge[0]], x2 = x[edge[1]]                       # gather  [E, IN]
    row = BN(x1 @ W.T + b), col = BN(x2 @ W.T + b)         # linear + global-batch BN
    out[a, b] = sigmoid(row_a^T  diag(D_b) R diag(D_b)  col_a)

Sharding: data-parallel over E across 8 cores (E_s = E/8 = 4096 per core).
x / weights / R / D replicated.  BatchNorm statistics are global over E:
each core computes per-feature partials (x-sum per side + sum of y^2 per
side), packed as a [128,4] tile; an AllGather (cheaper than AllReduce in
both the cost model and on the wire) + local tree-reduce produces the
global stats.  The y-sum is recovered as W @ xsum + E*b (exact).

Device layout is feature-major ([128 features on partitions, edges on the
free dim]): the linear and the 16 DEDICOM matmuls contract features on
the PE, BN stats are free-axis reductions, and BN application is a
per-partition scale/bias.  Gathers are batched 4 blocks per indirect DMA
(the v1 DMA cost has a 500ns/instruction floor).  Gathered edge-major
tiles are transposed on the PE with a bf16 identity (1 cycle/row).  The
final per-edge dot (sum_i row*u) is an elementwise multiply (split
DVE / ACT+GPSIMD lanes) + a PE "selector" matmul ([128,16] one-hot
column b) accumulating all 16 relations into one [16, 512] PSUM tile.
Output is produced relation-major [16, E_s]; the host transposes while
unsharding.
"""

import sys

sys.path.insert(0, "/opt/trn_rl_repo")

import numpy as np

import concourse.bass as bass
import concourse.tile as tile
from concourse import bacc, mybir
from concourse.bass_utils import run_bass_kernel_spmd

# Problem sizes (hardcoded per contract)
N_NODES = 50000
E = 32768
IN_DIM = 128
HID = 128
OUT = 16
EPS = 1e-5
N_CORES = 8
E_S = E // N_CORES          # 4096 edges per core
J = E_S // 128              # 32 gather blocks per side
NCH = E_S // 512            # 8 free-dim chunks of 512

F32 = mybir.dt.float32
F32R = mybir.dt.float32r
BF16 = mybir.dt.bfloat16

# tuning knobs
N_WARM = 82        # PE keep-warm matmuls spanning the collective bubble
CC_ALLGATHER = True  # AllGather+local reduce (False: plain AllReduce)
G = 5              # u/z software-pipeline depth ahead of the o-matmuls
POOL_LANES = frozenset(range(4, 12))  # z-lanes routed ACT-copy + GPSIMD-mul


def _build(centered=False):
    nc = bacc.Bacc(None, target_bir_lowering=False, debug=False, num_devices=N_CORES)

    # ---- I/O ----
    x = nc.dram_tensor("x", [N_NODES, IN_DIM], F32R, kind="ExternalInput")
    idx1 = nc.dram_tensor("idx1", [128, J], mybir.dt.int32, kind="ExternalInput")
    idx2 = nc.dram_tensor("idx2", [128, J], mybir.dt.int32, kind="ExternalInput")
    w_t = nc.dram_tensor("w_t", [IN_DIM, HID], F32R, kind="ExternalInput")
    r_t = nc.dram_tensor("r_t", [HID, HID], F32, kind="ExternalInput")
    d_m = nc.dram_tensor("d_m", [OUT, HID], F32, kind="ExternalInput")
    d_t = nc.dram_tensor("d_t", [HID, OUT], F32, kind="ExternalInput")
    lin_b = nc.dram_tensor("lin_b", [HID, 1], F32, kind="ExternalInput")
    gamma = nc.dram_tensor("gamma", [HID, 1], F32, kind="ExternalInput")
    beta = nc.dram_tensor("beta", [HID, 1], F32, kind="ExternalInput")
    ident = nc.dram_tensor("ident", [128, 128], F32, kind="ExternalInput")
    sel = nc.dram_tensor("sel", [128, OUT, OUT], F32, kind="ExternalInput")
    out = nc.dram_tensor("out", [OUT, E_S], F32, kind="ExternalOutput")

    with tile.TileContext(nc) as tc:
        with (
            tc.tile_pool(name="dramp", bufs=1, space="DRAM") as dramp,
            tc.tile_pool(name="consts", bufs=1) as consts,
            tc.tile_pool(name="gat", bufs=8) as gat,
            tc.tile_pool(name="big", bufs=1) as big,
            tc.tile_pool(name="zs", bufs=6) as zs,
            tc.tile_pool(name="small", bufs=2) as small,
            tc.tile_pool(name="outp", bufs=2) as outp,
            tc.tile_pool(name="psU", bufs=5, space="PSUM") as psU,
            tc.tile_pool(name="psO", bufs=3, space="PSUM") as psO,
        ):
            # ---- constants ----
            # idx first: the gather stream is the front-phase critical path
            idx1_s = consts.tile([128, J], mybir.dt.int32)
            nc.sync.dma_start(out=idx1_s[:], in_=idx1[:])
            idx2_s = consts.tile([128, J], mybir.dt.int32)
            nc.sync.dma_start(out=idx2_s[:], in_=idx2[:])
            w_t_s = consts.tile([IN_DIM, HID], F32R)
            nc.sync.dma_start(out=w_t_s[:], in_=w_t[:])
            ident_s = consts.tile([128, 128], F32)
            nc.sync.dma_start(out=ident_s[:], in_=ident[:])
            r_t_s = consts.tile([HID, HID], F32)
            nc.sync.dma_start(out=r_t_s[:], in_=r_t[:])
            d_t_s = consts.tile([HID, OUT], F32)
            nc.sync.dma_start(out=d_t_s[:], in_=d_t[:])
            sel_s = consts.tile([128, OUT, OUT], F32)
            nc.sync.dma_start(out=sel_s[:], in_=sel[:])
            lin_b_s = consts.tile([HID, 1], F32)
            nc.sync.dma_start(out=lin_b_s[:], in_=lin_b[:])
            gamma_s = consts.tile([HID, 1], F32)
            nc.sync.dma_start(out=gamma_s[:], in_=gamma[:])
            beta_s = consts.tile([HID, 1], F32)
            nc.sync.dma_start(out=beta_s[:], in_=beta[:])
            # D broadcast across partitions: dbc[p, b, i] = D[b, i]
            dbc_s = consts.tile([128, OUT, HID], F32)
            nc.sync.dma_start(
                out=dbc_s[:],
                in_=bass.AP(tensor=d_m, offset=0, ap=[[0, 128], [HID, OUT], [1, HID]]),
            )
            eps_s = consts.tile([HID, 1], F32)
            nc.vector.memset(eps_s[:], EPS)
            # f32r identity: the transpose's moving operand -> 1.5 cycles/row
            # (bf16 would be 1.0 but neuronxcc rejects mixed 32/16-bit matmuls)
            ident_b = consts.tile([128, 128], F32R)
            nc.vector.tensor_copy(out=ident_b[:], in_=ident_s[:])
            # fp32r-rounded copy of the selector weights
            sel_r = consts.tile([128, OUT, OUT], F32R)
            nc.vector.tensor_copy(out=sel_r[:], in_=sel_s[:])

            s_all = big.tile([128, OUT, HID], F32R, tag="s_all")

            def emit_s_all():
                # S_b^T tiles: s_all[f, b, j] = R[j,f] * D[b,j] * D[b,f].
                # Pure constants, but emitted on GPSIMD *after* the gather
                # stream so they can't steal front-end DVE/Pool bandwidth;
                # they complete long before the collective returns.
                # dbd[f, b, j] = D[b, f] * D[b, j] (in-place over dbc)
                nc.gpsimd.tensor_tensor(
                    out=dbc_s[:],
                    in0=dbc_s[:],
                    in1=bass.AP(
                        tensor=d_t_s.tensor,
                        offset=d_t_s[:].offset,
                        ap=[d_t_s[:].ap[0], [1, OUT], [0, HID]],
                    ),
                    op=mybir.AluOpType.mult,
                )
                nc.gpsimd.tensor_tensor(
                    out=s_all[:],
                    in0=bass.AP(
                        tensor=r_t_s.tensor,
                        offset=r_t_s[:].offset,
                        ap=[r_t_s[:].ap[0], [0, OUT], [1, HID]],
                    ),
                    in1=dbc_s[:],
                    op=mybir.AluOpType.mult,
                )

            # ---- per-side gather + transpose + linear + stats ----
            # The gather stream (64 single-offset indirect DMAs, 500ns floor
            # each on the Pool queue) is the front-end pacer; all per-chunk
            # elementwise work fits underneath it.  x-sums accumulate on the
            # ACT transpose-copies (y-sum is recovered post-collective as
            # W @ xsum_g + E*b); y^2 sums via ACT Square reading the linear's
            # PSUM with the bias folded in (lookbehind-1: zero queue stall).
            yTs = []
            xTs = []
            xs_parts = []
            sq_parts = []
            for side, idx_s in ((0, idx1_s), (1, idx2_s)):
                xT = big.tile([128, E_S], F32R, tag=f"xT{side}")
                yT = big.tile([128, E_S], F32, tag=f"yT{side}")
                xs_part = small.tile([128, NCH], F32, tag=f"xs{side}")
                q_part = small.tile([128, NCH], F32, tag=f"sq{side}")
                yps = []

                def emit_square(n, side=side, yps=None, q_part=q_part):
                    sq = zs.tile([128, 512], F32, tag="sq_scratch")
                    nc.scalar.activation(
                        out=sq[:],
                        in_=yps[n][:],
                        func=mybir.ActivationFunctionType.Square,
                        bias=lin_b_s[:, 0:1],
                        scale=1.0,
                        accum_out=q_part[:, n : n + 1],
                    )

                for n in range(NCH):  # 4 gathers == one 512 chunk
                    g4 = gat.tile([128, 4, 128], F32R, tag="g")
                    for k in range(4):
                        nc.gpsimd.indirect_dma_start(
                            out=g4[:, k, :],
                            out_offset=None,
                            in_=x[:],
                            in_offset=bass.IndirectOffsetOnAxis(
                                ap=idx_s[:, 4 * n + k : 4 * n + k + 1],
                                axis=0,
                            ),
                        )
                    tp4 = psU.tile([128, 4, 128], F32R, tag="u")
                    for k in range(4):
                        nc.tensor.transpose(
                            out=tp4[:, k, :], in_=g4[:, k, :], identity=ident_b[:]
                        )
                    sl = slice(n * 512, (n + 1) * 512)
                    # PSUM->SBUF with free-axis x-sum accumulation
                    nc.scalar.activation(
                        out=xT[:, sl],
                        in_=tp4[:],
                        func=mybir.ActivationFunctionType.Copy,
                        accum_out=xs_part[:, n : n + 1],
                    )
                    yp = psU.tile([128, 512], F32, tag="u")
                    nc.tensor.matmul(
                        out=yp[:], lhsT=w_t_s[:], rhs=xT[:, sl], start=True, stop=True
                    )
                    # psum -> sbuf with bias add (DVE)
                    nc.vector.tensor_scalar_add(
                        out=yT[:, sl], in0=yp[:], scalar1=lin_b_s[:, 0:1]
                    )
                    if n >= 1:
                        emit_square(n - 1, yps=yps)
                    yps.append(yp)
                yTs.append(yT)
                xTs.append(xT)
                xs_parts.append(xs_part)
                sq_parts.append(q_part)
                emit_square(NCH - 1, yps=yps)

            # constant S-tile build on GPSIMD after the gather stream drains
            emit_s_all()

            # ---- pack partial stats + collective ----
            # layout: [xsum0, xsum1, ysq0, ysq1] so the BN finalize can
            # process both sides with 2-column vector ops
            stats_l = small.tile([128, 4], F32, tag="stats")
            for k, part in (
                (0, xs_parts[0]),
                (1, xs_parts[1]),
                (2, sq_parts[0]),
                (3, sq_parts[1]),
            ):
                nc.vector.reduce_sum(
                    out=stats_l[:, k : k + 1],
                    in_=part[:],
                    axis=mybir.AxisListType.X,
                    op=mybir.AluOpType.add,
                )
            cc_in = dramp.tile([HID, 4], F32)
            if CC_ALLGATHER:
                cc_out = dramp.tile([N_CORES, HID, 4], F32, addr_space="Shared")
            else:
                cc_out = dramp.tile([HID, 4], F32, addr_space="Shared")
            nc.sync.dma_start(out=cc_in[:], in_=stats_l[:])
            # pre-load ACT tables for funcs first used after the collective
            # (Sigmoid / Rsqrt) while ACT idles in the bubble
            dum = small.tile([128, 1], F32, tag="dum")
            nc.scalar.activation(
                out=dum[:], in_=eps_s[:], func=mybir.ActivationFunctionType.Sigmoid
            )
            nc.scalar.activation(
                out=dum[:], in_=eps_s[:], func=mybir.ActivationFunctionType.Sqrt
            )
            if CC_ALLGATHER:
                nc.gpsimd.collective_compute(
                    "AllGather",
                    mybir.AluOpType.bypass,
                    replica_groups=[list(range(N_CORES))],
                    ins=[cc_in[:]],
                    outs=[cc_out[:]],
                )
                stats_r = small.tile([128, N_CORES, 4], F32, tag="statsr")
                nc.sync.dma_start(
                    out=stats_r[:],
                    in_=bass.AP(
                        tensor=cc_out.tensor,
                        offset=cc_out[:].offset,
                        ap=[[4, 128], [HID * 4, N_CORES], [1, 4]],
                    ),
                )
                # reduce the 8 ranks' partials in one strided op: view the
                # [128, 8, 4] tile as [128, 4, 8] (k outer / rank inner) so
                # the X-axis reduction sums over ranks
                stats_g = small.tile([128, 4], F32, tag="statsg")
                nc.vector.reduce_sum(
                    out=stats_g[:],
                    in_=bass.AP(
                        tensor=stats_r.tensor,
                        offset=stats_r[:].offset,
                        ap=[stats_r[:].ap[0], [1, 4], [4, N_CORES]],
                    ),
                    axis=mybir.AxisListType.X,
                    op=mybir.AluOpType.add,
                )
            else:
                nc.gpsimd.collective_compute(
                    "AllReduce",
                    mybir.AluOpType.add,
                    replica_groups=[list(range(N_CORES))],
                    ins=[cc_in[:]],
                    outs=[cc_out[:]],
                )
                stats_g = small.tile([128, 4], F32, tag="statsg")
                nc.sync.dma_start(out=stats_g[:], in_=cc_out[:])

            # PE keep-warm across the collective: WAW-serialized dummy
            # matmuls so the clock stays at full rate for the post-collective
            # DEDICOM matmuls
            warm_ps = psU.tile([128, 512], F32, tag="u")
            for _k in range(N_WARM):
                nc.tensor.matmul(
                    out=warm_ps[:], lhsT=w_t_s[:], rhs=xTs[0][:, 0:512],
                    start=True, stop=True,
                )

            # ---- finalize BN factors (both sides as [128, 2] vector ops) ----
            # means = (W @ xsum_g)/E + b   (exact: the linear is affine)
            inv_e = 1.0 / float(E)
            xsum2 = small.tile([128, 2], F32R, tag="xs2")
            nc.vector.tensor_copy(out=xsum2[:], in_=stats_g[:, 0:2])
            ysum_big = psU.tile([128, 512], F32, tag="u")
            nc.tensor.matmul(
                out=ysum_big[:, 0:2], lhsT=w_t_s[:], rhs=xsum2[:],
                start=True, stop=True,
            )
            means = small.tile([128, 2], F32, tag="means")
            nc.vector.scalar_tensor_tensor(
                out=means[:],
                in0=ysum_big[:, 0:2],
                scalar=inv_e,
                in1=bass.AP(
                    tensor=lin_b_s.tensor,
                    offset=lin_b_s[:].offset,
                    ap=[lin_b_s[:].ap[0], [0, 2]],
                ),
                op0=mybir.AluOpType.mult,
                op1=mybir.AluOpType.add,
            )
            sgs = small.tile([128, 2], F32, tag="sgs")  # [ey2_0, ey2_1]
            nc.scalar.mul(out=sgs[:], in_=stats_g[:, 2:4], mul=inv_e)
            m2 = small.tile([128, 2], F32, tag="m2")
            nc.vector.tensor_tensor(
                out=m2[:], in0=means[:], in1=means[:], op=mybir.AluOpType.mult
            )
            var2 = small.tile([128, 2], F32, tag="var2")
            nc.vector.tensor_sub(out=var2[:], in0=sgs[:], in1=m2[:])
            std2 = small.tile([128, 2], F32, tag="std2")
            nc.scalar.activation(
                out=std2[:],
                in_=var2[:],
                func=mybir.ActivationFunctionType.Sqrt,
                bias=eps_s[:, 0:1],
                scale=1.0,
            )
            rstd = small.tile([128, 2], F32, tag="rstd")
            nc.vector.reciprocal(out=rstd[:], in_=std2[:])
            bn_sc = small.tile([128, 2], F32, tag="bnsc")
            nc.vector.tensor_scalar_mul(
                out=bn_sc[:], in0=rstd[:], scalar1=gamma_s[:, 0:1]
            )
            msc = small.tile([128, 2], F32, tag="msc")
            nc.vector.tensor_tensor(
                out=msc[:], in0=means[:], in1=bn_sc[:], op=mybir.AluOpType.mult
            )
            bn_sh = small.tile([128, 2], F32, tag="bnsh")
            nc.vector.tensor_scalar(
                out=bn_sh[:],
                in0=msc[:],
                scalar1=-1.0,
                scalar2=beta_s[:, 0:1],
                op0=mybir.AluOpType.mult,
                op1=mybir.AluOpType.add,
            )
            bn_s = [bn_sc[:, 0:1], bn_sc[:, 1:2]]
            bn_t = [bn_sh[:, 0:1], bn_sh[:, 1:2]]

            # ---- apply BN (feature-major: per-partition scale+shift) ----
            # emitted just-in-time inside the DEDICOM loop (one chunk of
            # lookahead) so the 8-chunk BN batch can't head-of-line-block the
            # DVE/ACT queues at back-end start
            rowT = big.tile([128, E_S], F32R, tag="rowT")
            colT = big.tile([128, E_S], F32R, tag="colT")

            def emit_bn(n):
                sl = slice(n * 512, (n + 1) * 512)
                nc.vector.tensor_scalar(
                    out=colT[:, sl],
                    in0=yTs[1][:, sl],
                    scalar1=bn_s[1],
                    scalar2=bn_t[1],
                    op0=mybir.AluOpType.mult,
                    op1=mybir.AluOpType.add,
                )
                nc.scalar.activation(
                    out=rowT[:, sl],
                    in_=yTs[0][:, sl],
                    func=mybir.ActivationFunctionType.Identity,
                    bias=bn_t[0],
                    scale=bn_s[0],
                )

            emit_bn(0)

            # ---- DEDICOM: u_b = S_b^T col ; z = row*u ; o_b = sum_i z ----
            # one flat (chunk, b) pipeline: u-matmuls + z-muls run G steps
            # ahead of the strictly-ordered o-accumulation matmuls, ACROSS
            # chunk boundaries, so the z latency (DVE/ACT/GPSIMD) stays off
            # the PE's critical path even at chunk seams.
            op_tiles = [None] * NCH
            ztiles = [[None] * OUT for _ in range(NCH)]

            def emit_u_z(n, b):
                sl = slice(n * 512, (n + 1) * 512)
                if b == 0:
                    if n + 1 < NCH:
                        emit_bn(n + 1)
                    op_t = psO.tile([OUT, 512], F32, tag="o")
                    op_tiles[n] = op_t
                up = psU.tile([128, 512], F32, tag="u")
                nc.tensor.matmul(
                    out=up[:],
                    lhsT=s_all[:, b, :],
                    rhs=colT[:, sl],
                    start=True,
                    stop=True,
                )
                z = zs.tile([128, 512], F32R, tag="z")
                if b in POOL_LANES:
                    # ACT copies PSUM->SBUF, GPSIMD muls
                    u_sb = zs.tile([128, 512], F32, tag="usb")
                    nc.scalar.copy(out=u_sb[:], in_=up[:])
                    nc.gpsimd.tensor_tensor(
                        out=z[:],
                        in0=u_sb[:],
                        in1=rowT[:, sl],
                        op=mybir.AluOpType.mult,
                    )
                else:
                    nc.vector.tensor_tensor(
                        out=z[:],
                        in0=up[:],
                        in1=rowT[:, sl],
                        op=mybir.AluOpType.mult,
                    )
                ztiles[n][b] = z

            def emit_o(n, b):
                nc.tensor.matmul(
                    out=op_tiles[n][:],
                    lhsT=sel_r[:, b, :],
                    rhs=ztiles[n][b][:],
                    start=(b == 0),
                    stop=(b == OUT - 1),
                )
                ztiles[n][b] = None

            def emit_sig(n):
                sl = slice(n * 512, (n + 1) * 512)
                o_sb = outp.tile([OUT, 512], F32, tag="osb")
                nc.scalar.activation(
                    out=o_sb[:],
                    in_=op_tiles[n][:],
                    func=mybir.ActivationFunctionType.Sigmoid,
                )
                nc.sync.dma_start(out=out[:, sl], in_=o_sb[:])

            work = [(n, b) for n in range(NCH) for b in range(OUT)]
            for i, (n, b) in enumerate(work):
                emit_u_z(n, b)
                if i >= G:
                    emit_o(*work[i - G])
                # sigmoid for the previous chunk is emitted a few u_sb copies
                # into this one, so it can't head-of-line-block the ACT queue
                # while it waits on the previous chunk's last o-matmul
                if b == 8 and n > 0:
                    emit_sig(n - 1)
            for i in range(len(work) - G, len(work)):
                emit_o(*work[i])
            emit_sig(NCH - 1)

    nc.compile()
    return nc


_CACHE = {}


def _get_nc():
    if "nc" not in _CACHE:
        _CACHE["nc"] = _build()
    return _CACHE["nc"]


def _marshal(x, target_edge_index, lin_w, lin_b, bn_gamma, bn_beta, R, D):
    x = np.ascontiguousarray(np.asarray(x, dtype=np.float32))
    edges = np.asarray(target_edge_index)
    sel = np.zeros((128, OUT, OUT), dtype=np.float32)
    for b in range(OUT):
        sel[:, b, b] = 1.0
    common = {
        "w_t": np.ascontiguousarray(np.asarray(lin_w, np.float32).T),
        "r_t": np.ascontiguousarray(np.asarray(R, np.float32).T),
        "d_m": np.ascontiguousarray(np.asarray(D, np.float32)),
        "d_t": np.ascontiguousarray(np.asarray(D, np.float32).T),
        "lin_b": np.ascontiguousarray(np.asarray(lin_b, np.float32).reshape(HID, 1)),
        "gamma": np.ascontiguousarray(np.asarray(bn_gamma, np.float32).reshape(HID, 1)),
        "beta": np.ascontiguousarray(np.asarray(bn_beta, np.float32).reshape(HID, 1)),
        "ident": np.eye(128, dtype=np.float32),
        "sel": sel,
        "x": x,
    }
    in_maps = []
    for c in range(N_CORES):
        sl = slice(c * E_S, (c + 1) * E_S)
        i1 = edges[0, sl].astype(np.int32).reshape(J, 128).T
        i2 = edges[1, sl].astype(np.int32).reshape(J, 128).T
        in_maps.append(
            {**common, "idx1": np.ascontiguousarray(i1), "idx2": np.ascontiguousarray(i2)}
        )
    return in_maps


def kernel(x, target_edge_index, lin_w, lin_b, bn_gamma, bn_beta, R, D):
    nc = _get_nc()
    in_maps = _marshal(x, target_edge_index, lin_w, lin_b, bn_gamma, bn_beta, R, D)
    _CACHE["in_maps"] = in_maps
    res = run_bass_kernel_spmd(nc, in_maps, list(range(N_CORES)))
    shards = [res.results[c]["out"] for c in range(N_CORES)]  # each [16, E_S]
    full = np.concatenate(shards, axis=1)  # [16, E]
    return np.ascontiguousarray(full.T)  # [E, 16] float32
